# revision 4
# baseline (speedup 1.0000x reference)
"""Trainium2 Bass kernel for a fused GRUCell step.

Math (reference):
    xi = x @ [W_ir W_iz W_in] + [b_ir b_iz b_in]
    hh = h @ [W_hr W_hz W_hn]
    r = sigmoid(xr + hr); z = sigmoid(xz + hz)
    n = tanh(xn + r * (hn + b_hn))
    new_h = (1 - z) * n + z * h

Strategy: pure data-parallel over the batch dim (B=16384 -> 8 cores x 2048).
Weights are replicated. Per core, one K-concatenated GEMM family with
K = F + H = 2048: lhsT = [x_shard; h_shard]^T (fp16), rhs = per-gate
[W_i*; W_h*] concat (fp16). The r and z gates accumulate x- and h-products
into the same PSUM bank (K=2048); the n gate keeps xn and hn separate
(the recurrence multiplies hn by r before adding xn). Elementwise gates run
on ScalarE (sigmoid/tanh) + VectorE (mul/add/sub), fp32 throughout.

Perf notes (v2): the GEMM stream itself runs gapless at ~334us (fp16 PE
roofline is ~328us); the optimization targets are the edges:
  - startup: weights (12.6MB) + first lhsT block used to arrive via a few
    huge DMAs, so the first matmul waited ~48us after kernel start. Now all
    DRAM operands are host-packed to match SBUF tile layout (contiguous
    per-partition lines) and DMAed in consumption order in 2-ko slices, so
    matmuls start ~2us in, paced by DMA arrival.
  - first block is hc-major (all 4 row-blocks of H-chunk 0, then chunk 1) so
    it only needs half the weights before useful work.
  - ~16 warmup matmuls on a zeroed dummy tile run during the initial DMA
    window so the PE's HAM clock-gate reaches 8/8 before the real stream.
  - h ships as fp16 (only used in the z*h elementwise term; error ~5e-5).
"""

import os
import sys

import numpy as np

sys.path.insert(0, "/opt/trn_rl_repo")
os.environ.setdefault("MYCRO_LOCAL_CACHE", "1")

import concourse.bass as bass  # noqa: E402
import concourse.mybir as mybir  # noqa: E402
import concourse.tile as tile  # noqa: E402
from concourse import bacc  # noqa: E402
from concourse.bass_utils import run_bass_kernel_spmd  # noqa: E402

N_CORES = 8
F = 1024  # input feature dim
H = 1024  # hidden dim
K = F + H  # GEMM contraction dim (x features then h features)
P = 128
KO = K // P  # 16 k-chunks of 128
KOX = F // P  # 8 k-chunks belonging to the x part
MBLK = 512  # batch rows staged per lhsT block
NC_CHUNK = 512  # H columns per PSUM bank / matmul
KG = 2  # k-chunks per fine-grained startup DMA slice
GATES = ("r", "z", "n")


def build_gru_program(b_core: int, with_bias: bool) -> bass.Bass:
    """One SPMD program; every core runs it on its own batch shard."""
    fp16 = mybir.dt.float16
    f32 = mybir.dt.float32
    n_blk = b_core // MBLK
    assert b_core % MBLK == 0
    hc_n = H // NC_CHUNK
    ms_n = MBLK // P

    # Bacc (not plain Bass): its compile pipeline splits multi-sem waits into
    # event semaphores — walrus rejects >1 wait on most engine instructions.
    nc = bacc.Bacc()
    lhsT = nc.declare_dram_parameter("lhsT", [n_blk, P, KO, MBLK], fp16, isOutput=False)
    wparams = {
        (g, hc): nc.declare_dram_parameter(f"w{g}{hc}", [P, KO, NC_CHUNK], fp16, isOutput=False)
        for g in GATES
        for hc in range(hc_n)
    }
    h16 = nc.declare_dram_parameter("h16", [b_core, H], fp16, isOutput=False)
    if with_bias:
        # host-replicated across partitions; rows: b_r, b_z, b_in, b_hn
        bias = nc.declare_dram_parameter("bias_rep", [P, 4, H], f32, isOutput=False)
    out = nc.declare_dram_parameter("out", [b_core, H], f32, isOutput=True)

    Sigmoid = mybir.ActivationFunctionType.Sigmoid
    Tanh = mybir.ActivationFunctionType.Tanh

    with tile.TileContext(nc) as tc:
        with (
            tc.tile_pool(name="wpool", bufs=1) as wpool,
            tc.tile_pool(name="lpool", bufs=2) as lpool,
            tc.tile_pool(name="hpool", bufs=2) as hpool,
            tc.tile_pool(name="opool", bufs=3) as opool,
            tc.tile_pool(name="epool", bufs=2) as epool,
            tc.tile_pool(name="psum", bufs=2, space="PSUM") as psum,
        ):
            # PE warmup: the HAM clock gate keeps the PE at 1.2GHz until it
            # has seen ~3.4us of CONTINUOUS activity. Burn one whole window on
            # a dense dummy-matmul burst while the first weight slices stream
            # in; the DMA-paced ramp that follows has gaps <3.4us, so the PE
            # stays at 2.4GHz once unthrottled.
            junk = wpool.tile([P, P], fp16, tag="junk")
            nc.vector.memset(junk[:], 0)
            warm_ps = psum.tile([P, NC_CHUNK], f32, tag="pr")
            for _ in range(36):
                nc.tensor.matmul(warm_ps[:, :P], junk[:], junk[:], start=True, stop=True)

            # Resident weights; fine-grained DMAs in the exact order the
            # first block's matmuls consume them: per k-slice, the ms0
            # quarter of the lhsT block plus the three hc0 weight slices
            # (everything row-block ms0 of H-chunk 0 needs), then the lhsT
            # remainders, h rows, and finally the hc1 weights.
            wsb = {
                (g, hc): wpool.tile(
                    [P, KO, NC_CHUNK], fp16, tag=f"w{g}{hc}", name=f"w{g}{hc}"
                )
                for g in GATES
                for hc in range(hc_n)
            }
            lt = lpool.tile([P, KO, MBLK], fp16, tag="lhsT")
            ht = {}
            for kg in range(KO // KG):
                ks = slice(kg * KG, (kg + 1) * KG)
                nc.sync.dma_start(lt[:, ks, :P], lhsT[0, :, ks, :P])
                for g in GATES:
                    nc.sync.dma_start(wsb[(g, 0)][:, ks, :], wparams[(g, 0)][:, ks, :])
            for kg in range(KO // KG):
                ks = slice(kg * KG, (kg + 1) * KG)
                nc.sync.dma_start(lt[:, ks, P:], lhsT[0, :, ks, P:])
            for ms in range(ms_n):
                t = hpool.tile([P, H], fp16, tag=f"h{ms}")
                ht[ms] = t
                nc.sync.dma_start(t[:], h16[ms * P : (ms + 1) * P, :])
            for kg in range(KO // KG):
                ks = slice(kg * KG, (kg + 1) * KG)
                for g in GATES:
                    for hc in range(1, hc_n):
                        nc.sync.dma_start(wsb[(g, hc)][:, ks, :], wparams[(g, hc)][:, ks, :])

            bias_sb = None
            if with_bias:
                bias_sb = wpool.tile([P, 4, H], f32, tag="bias_sb")
                nc.sync.dma_start(bias_sb[:], bias[:])

            for blk in range(n_blk):
                if blk > 0:
                    lt = lpool.tile([P, KO, MBLK], fp16, tag="lhsT")
                    nc.sync.dma_start(lt[:], lhsT[blk])
                    for ms in range(ms_n):
                        t = hpool.tile([P, H], fp16, tag=f"h{ms}")
                        ht[ms] = t
                        m0 = blk * MBLK + ms * P
                        nc.sync.dma_start(t[:], h16[m0 : m0 + P, :])
                for hc in range(hc_n):
                    cs = slice(hc * NC_CHUNK, (hc + 1) * NC_CHUNK)
                    for ms in range(ms_n):
                        m0 = blk * MBLK + ms * P
                        pr = psum.tile([P, NC_CHUNK], f32, tag="pr")
                        pz = psum.tile([P, NC_CHUNK], f32, tag="pz")
                        pxn = psum.tile([P, NC_CHUNK], f32, tag="pxn")
                        phn = psum.tile([P, NC_CHUNK], f32, tag="phn")
                        for ko in range(KO):
                            ls = lt[:, ko, ms * P : (ms + 1) * P]
                            nc.tensor.matmul(
                                pr[:],
                                ls,
                                wsb[("r", hc)][:, ko, :],
                                start=(ko == 0),
                                stop=(ko == KO - 1),
                            )
                            nc.tensor.matmul(
                                pz[:],
                                ls,
                                wsb[("z", hc)][:, ko, :],
                                start=(ko == 0),
                                stop=(ko == KO - 1),
                            )
                            if ko < KOX:
                                nc.tensor.matmul(
                                    pxn[:],
                                    ls,
                                    wsb[("n", hc)][:, ko, :],
                                    start=(ko == 0),
                                    stop=(ko == KOX - 1),
                                )
                            else:
                                nc.tensor.matmul(
                                    phn[:],
                                    ls,
                                    wsb[("n", hc)][:, ko, :],
                                    start=(ko == KOX),
                                    stop=(ko == KO - 1),
                                )

                        sr = epool.tile([P, NC_CHUNK], f32, tag="sr")
                        sz = epool.tile([P, NC_CHUNK], f32, tag="sz")
                        sn = epool.tile([P, NC_CHUNK], f32, tag="sn")
                        tt = epool.tile([P, NC_CHUNK], f32, tag="tt")
                        if with_bias:
                            nc.vector.tensor_add(tt[:], pr[:], bias_sb[:, 0, cs])
                            nc.scalar.activation(sr[:], tt[:], Sigmoid)
                            nc.vector.tensor_add(tt[:], pz[:], bias_sb[:, 1, cs])
                            nc.scalar.activation(sz[:], tt[:], Sigmoid)
                            nc.vector.tensor_add(tt[:], phn[:], bias_sb[:, 3, cs])
                            nc.vector.tensor_mul(tt[:], sr[:], tt[:])
                            nc.vector.tensor_add(tt[:], tt[:], pxn[:])
                            nc.vector.tensor_add(tt[:], tt[:], bias_sb[:, 2, cs])
                        else:
                            nc.scalar.activation(sr[:], pr[:], Sigmoid)
                            nc.scalar.activation(sz[:], pz[:], Sigmoid)
                            nc.vector.tensor_mul(tt[:], sr[:], phn[:])
                            nc.vector.tensor_add(tt[:], tt[:], pxn[:])
                        nc.scalar.activation(sn[:], tt[:], Tanh)
                        ot = opool.tile([P, NC_CHUNK], f32, tag="ot")
                        nc.vector.tensor_sub(tt[:], ht[ms][:, cs], sn[:])
                        nc.vector.tensor_mul(tt[:], tt[:], sz[:])
                        nc.vector.tensor_add(ot[:], sn[:], tt[:])
                        nc.sync.dma_start(out[m0 : m0 + P, cs], ot[:])
    nc.finalize()
    return nc


_PROGRAM_CACHE: dict = {}


def get_program(b_core: int, with_bias: bool) -> bass.Bass:
    key = (b_core, with_bias)
    if key not in _PROGRAM_CACHE:
        _PROGRAM_CACHE[key] = build_gru_program(b_core, with_bias)
    return _PROGRAM_CACHE[key]


def prepare_in_maps(h, x, W_ir, W_iz, W_in, b_ir, b_iz, b_in, W_hr, W_hz, W_hn, b_hn):
    """Host-side shard + layout prep. Returns (in_maps, with_bias, b_core)."""
    h = np.ascontiguousarray(np.asarray(h, dtype=np.float32))
    x = np.ascontiguousarray(np.asarray(x, dtype=np.float32))
    b_full = x.shape[0]
    assert b_full % N_CORES == 0
    b_core = b_full // N_CORES
    n_blk = b_core // MBLK
    hc_n = H // NC_CHUNK

    # Weights packed to the SBUF tile layout [p, ko, n-chunk] so each DMA
    # slice is contiguous per partition.
    wmats = {}
    for g, wi, wh in (("r", W_ir, W_hr), ("z", W_iz, W_hz), ("n", W_in, W_hn)):
        wk = np.concatenate([wi, wh], axis=0).astype(np.float16)  # [K, H]
        wk = wk.reshape(KO, P, H).transpose(1, 0, 2)  # [p, ko, H]
        for hc in range(hc_n):
            cs = slice(hc * NC_CHUNK, (hc + 1) * NC_CHUNK)
            wmats[f"w{g}{hc}"] = np.ascontiguousarray(wk[:, :, cs])

    br = np.asarray(b_ir, np.float32)
    bz = np.asarray(b_iz, np.float32)
    bn = np.asarray(b_in, np.float32)
    bhn = np.asarray(b_hn, np.float32)
    biases = np.stack([br, bz, bn, bhn]).astype(np.float32)
    with_bias = bool(np.any(biases != 0.0))

    in_maps = []
    for c in range(N_CORES):
        sl = slice(c * b_core, (c + 1) * b_core)
        xc = x[sl]
        hc = h[sl]
        lhsT_full = np.empty((K, b_core), np.float16)
        lhsT_full[:F] = xc.T
        lhsT_full[F:] = hc.T
        # [K, b_core] -> [blk, p, ko, m] so per-blk (and per-k-slice) DMAs
        # read contiguous per-partition lines.
        lhsT_t = np.ascontiguousarray(
            lhsT_full.reshape(KO, P, n_blk, MBLK).transpose(2, 1, 0, 3)
        )
        m = {
            "lhsT": lhsT_t,
            "h16": np.ascontiguousarray(hc.astype(np.float16)),
        }
        m.update(wmats)
        if with_bias:
            m["bias_rep"] = np.ascontiguousarray(
                np.broadcast_to(biases[None], (P, 4, H))
            )
        in_maps.append(m)
    return in_maps, with_bias, b_core


def kernel(h, x, W_ir, W_iz, W_in, b_ir, b_iz, b_in, W_hr, W_hz, W_hn, b_hn):
    in_maps, with_bias, b_core = prepare_in_maps(
        h, x, W_ir, W_iz, W_in, b_ir, b_iz, b_in, W_hr, W_hz, W_hn, b_hn
    )
    nc = get_program(b_core, with_bias)
    res = run_bass_kernel_spmd(nc, in_maps, list(range(N_CORES)))
    new_h = np.concatenate([res.results[c]["out"] for c in range(N_CORES)], axis=0)
    return (new_h, new_h)
